# revision 23
# baseline (speedup 1.0000x reference)
"""BitNet-style row-parallel linear on 8 TRN2 NeuronCores.

Reference computes: out[b,s,o] = sum_d x[b,s,d] * sign(w[o,d]) + bias[o]
  x: [4, 2048, 4096] f32, w: [4096, 4096] f32, bias: [4096] f32.

Strategy: data-parallel over the 8192 (b*s) rows — each of the 8 cores
computes a 1024-row slice of the output against the full binarized
weight. No collective needed; shards concatenate to the full output.

Default IMPL "fp8": the GEMM runs in fp8 e4m3 with DoubleRow perf mode
(2 fp8 MACs per PE cell per cycle; each N=512 matmul covers K=256 in
the same ~216ns a bf16 matmul spends on K=128 — LDWEIGHTS hides fully
under the moving stream). Weights are exactly +-1 in e4m3 so all
quantization error comes from x. Pure-e4m3 error (2.12e-2 rel) just
misses the 2e-2 gate, so the K dimension is extended with residual
correction columns: for the first CORR=768 input features,
fp8(x - fp8(x)) is appended as extra K rows whose +-1 weight rows
duplicate the originals, so the SBUF weight tiles are simply reused —
no extra weight HBM traffic. K' = 4864, measured rel err 1.884e-2
(deterministic: fixed seed, fp32 PSUM accumulation, bit-identical
across runs; numpy emulation matches HW to 0.02%).

TensorE consumes both operands K-major with the DoubleRow pair layout
[128 partitions, 2, free]: partition p, pair i holds logical k row
tile*256 + i*128 + p. The host pre-tiles both operands into exactly
this layout so every DMA is a flat contiguous per-partition copy.

Schedule (the PE stream measures gap-free at the 216ns/MM instruction
floor; total ~281us vs 464us for the bf16 baseline):
 - kxn (16MB) becomes fully SBUF-resident as 64 quarter-tiles;
   kxm streams in m-halves on separate DMA rings, ordered exactly as
   consumed (issue order != consumption order cost 11us of stalls).
 - Phase 1 (while kxn streams): nb {0,1} for m-quads, k outer in an
   order that interleaves residual k-tiles (whose kxn tiles arrived
   first) — paces fresh-byte demand under the ~345GB/s DMA supply.
 - Phase 2: per m-tile, per PSUM bank (nb 2..7), dense k loop;
   evictions (vector/scalar alternating) and output DMAs (3 rings)
   pipeline behind the next bank's matmuls.
No warmup: phase 1's first ~8 cold (1.2GHz) matmuls usefully slow
consumption while the HAM clock gate opens and kxn streams in.
"""

import numpy as np

B, S, D_IN, D_OUT = 4, 2048, 4096, 4096
NCORES = 8
M_TOTAL = B * S
M_CORE = M_TOTAL // NCORES

import os

_cache = {}

# fp8 config: pair-tiles cover 256 logical k each.
CORR = int(os.environ.get("BK_CORR", "768"))   # residual-corrected columns
KP = (D_IN + CORR) // 256                       # kxm pair-tiles
SP = D_IN // 256                                # kxn base pair-tiles (16)
assert (D_IN + CORR) % 256 == 0 and CORR // 256 <= SP

DTYPE = os.environ.get("BK_DTYPE", "bf16")      # for legacy bf16 impls
IMPL = os.environ.get("BK_IMPL", "fp8")


def _fp8_body(nc, tc, kxm, kxn, kxn00a, kxn00b, kxm00, out, mybir):
    P = 128
    MT = M_CORE // P        # 8 m tiles
    NW = 512
    NB = D_OUT // NW        # 8 n blocks
    RES = CORR // 256       # residual pair-tiles (reuse kxn tiles 0..RES-1)
    f32 = mybir.dt.float32
    f8 = mybir.dt.float8e4
    DR = mybir.MatmulPerfMode.DoubleRow

    from contextlib import ExitStack
    with ExitStack() as ctx:
        kxn_pool = ctx.enter_context(tc.tile_pool(name="kxn", bufs=4 * SP))
        kxm_pool = ctx.enter_context(tc.tile_pool(name="kxm", bufs=2 * KP))
        psum_pool = ctx.enter_context(
            tc.tile_pool(name="psum", bufs=8, space="PSUM"))
        out_pool = ctx.enter_context(tc.tile_pool(name="outp", bufs=8))

        # Tiny step-0 staging: the first matmuls' bytes duplicated as
        # ~128KB tensors, issued first on three different rings so the
        # PE starts ~2.5us earlier than the full first quarter-tile.
        t00a = kxn_pool.tile([P, 2, NW], f8, tag="k00", name="kxn00a",
                             bufs=3)
        nc.sync.dma_start(out=t00a, in_=kxn00a)
        t00b = kxn_pool.tile([P, 2, NW], f8, tag="k00", name="kxn00b",
                             bufs=3)
        nc.gpsimd.dma_start(out=t00b, in_=kxn00b)
        tm00 = kxn_pool.tile([P, 2, 256], f8, tag="k00", name="kxm00",
                             bufs=3)
        nc.scalar.dma_start(out=tm00, in_=kxm00)

        # kxn: 4 n-quarters x 16 base pair-tiles, all SBUF-resident
        # (2KB/partition each). Residual k rows reuse tiles 0..RES-1.
        # Issue order = first-needed order: quarter 0 first, so the
        # warm-up phase only waits on 256KB-granularity arrivals.
        kxn_tiles = {}
        for q in range(4):
            for sp in range(SP):
                t = kxn_pool.tile([P, 2, D_OUT // 4], f8, tag="kxn",
                                  name=f"kxn_{q}_{sp}", bufs=4 * SP)
                nc.sync.dma_start(out=t, in_=kxn[q, sp])
                kxn_tiles[(q, sp)] = t
        # Warm-up k order: residual tiles (which reuse the
        # earliest-arriving kxn tiles) interleaved among the first base
        # tiles to pace consumption of fresh DMA bytes.
        korder = []
        for k in range(KP):
            korder.append(k)
            if k < RES:
                korder.append(SP + k)
        korder += [k for k in range(RES, SP) if k not in korder]
        korder = korder[:KP]
        assert sorted(korder) == list(range(KP))

        # kxm: m-halves so the warm-up phase only pulls the rows it
        # uses. mh0 is issued in korder (the order P1a consumes it);
        # mh1 rides the otherwise-idle gpsimd ring so it doesn't
        # compete with mh0+kxn during the supply-bound warm-up.
        kxm_tiles = {}
        for mh, ks, eng in ((0, korder, nc.scalar), (1, range(KP), nc.gpsimd)):
            for k in ks:
                t = kxm_pool.tile([P, 2, M_CORE // 2], f8, tag="kxm",
                                  name=f"kxm_{k}_{mh}", bufs=2 * KP)
                eng.dma_start(out=t, in_=kxm[k, mh])
                kxm_tiles[(k, mh)] = t

        def lhsT(k, m):
            mh, off = divmod(m, MT // 2)
            return kxm_tiles[(k, mh)][:, :, off * P:(off + 1) * P]

        def rhs(sp, nb):
            q, j = divmod(nb, 2)
            return kxn_tiles[(q, sp)][:, :, j * NW:(j + 1) * NW]

        def evict(psum_t, m, nb, j):
            ot = out_pool.tile([P, NW], f32, tag="ot",
                               name=f"ot_{m}_{nb}", bufs=8)
            if j % 2 == 0:
                nc.vector.tensor_copy(out=ot[:, :], in_=psum_t[:, :])
            else:
                nc.scalar.copy(out=ot[:, :], in_=psum_t[:, :])
            if m == MT - 1 and nb >= NB - 2:
                h = NW // 2
                nc.gpsimd.dma_start(
                    out=out[m * P:(m + 1) * P, nb * NW:nb * NW + h],
                    in_=ot[:, :h])
                nc.sync.dma_start(
                    out=out[m * P:(m + 1) * P, nb * NW + h:(nb + 1) * NW],
                    in_=ot[:, h:])
            else:
                deng = (nc.gpsimd, nc.sync, nc.scalar)[j % 3]
                deng.dma_start(
                    out=out[m * P:(m + 1) * P, nb * NW:(nb + 1) * NW],
                    in_=ot[:, :])

        # Phase 1: nb {0,1} for all m, in two m-quads. 2x weight reuse,
        # paced so kxn/kxm DMA keeps up; P1b re-uses kxn while the rest
        # of kxn streams in behind it.
        for mq in range(2):
            ms = range(mq * 4, mq * 4 + 4)
            psums = {(m, nb): psum_pool.tile(
                [P, NW], f32, tag="ps", name=f"ps1_{m}_{nb}")
                for m in ms for nb in range(2)}
            for ki, k in enumerate(korder if mq == 0 else range(KP)):
                sp = k if k < SP else k - SP
                for m in ms:
                    if mq == 0 and ki == 0 and m < 2:
                        lt = tm00[:, :, m * P:(m + 1) * P]
                    else:
                        lt = lhsT(k, m)
                    for nb in range(2):
                        if mq == 0 and ki == 0:
                            rh = t00a if nb == 0 else t00b
                            rh = rh[:, :, :]
                        else:
                            rh = rhs(sp, nb)
                        nc.tensor.matmul(
                            psums[(m, nb)][:, :], lhsT=lt, rhs=rh,
                            start=(ki == 0), stop=(ki == KP - 1),
                            perf_mode=DR)
            for j, (m, nb) in enumerate(psums):
                evict(psums[(m, nb)], m, nb, j)

        # Phase 2: per m-tile, nb {2..7}, one bank at a time with its
        # own k-loop (LDWEIGHTS hides under every 512-wide matmul, so
        # reuse order is free): each bank's eviction+store pipelines
        # behind the next bank's matmuls, leaving only a ~2us tail.
        for m in range(MT):
            psums = [psum_pool.tile([P, NW], f32, tag="ps",
                                    name=f"ps2_{m}_{j}")
                     for j in range(NB - 2)]
            for j in range(NB - 2):
                for k in range(KP):
                    sp = k if k < SP else k - SP
                    nc.tensor.matmul(
                        psums[j][:, :], lhsT=lhsT(k, m), rhs=rhs(sp, j + 2),
                        start=(k == 0), stop=(k == KP - 1),
                        perf_mode=DR)
                evict(psums[j], m, j + 2, j)


def _custom_body(nc, tc, kxm, kxn, out, mm_dt, mybir):
    """Legacy bf16 path: x^T SBUF-resident; sign(w)^T streams through."""
    P = 128
    KT = D_IN // P          # 32 k tiles
    MT = M_CORE // P        # 8 m tiles
    NW = 512
    NB = D_OUT // NW        # 8 n blocks
    f32 = mybir.dt.float32

    from contextlib import ExitStack
    with ExitStack() as ctx:
        kxm_pool = ctx.enter_context(tc.tile_pool(name="kxm", bufs=1))
        kxn_pool = ctx.enter_context(tc.tile_pool(name="kxn", bufs=9))
        psum_pool = ctx.enter_context(
            tc.tile_pool(name="psum", bufs=8, space="PSUM"))
        out_pool = ctx.enter_context(tc.tile_pool(name="outp", bufs=8))

        def issue_chunk(nb, c, k0, sz):
            t = kxn_pool.tile([P, sz, NW], mm_dt, tag="kxn",
                              name=f"kxn_{nb}_{c}", bufs=24)
            src = kxn[k0 * P:(k0 + sz) * P, nb * NW:(nb + 1) * NW]
            nc.sync.dma_start(
                out=t, in_=src.rearrange("(ko ki) n -> ki ko n", ki=P))
            return [t[:, i, :] for i in range(sz)]

        def issue_chunks(nb, sizes):
            rhs, k0 = [], 0
            for c, sz in enumerate(sizes):
                rhs += issue_chunk(nb, c, k0, sz)
                k0 += sz
            return rhs

        kxm_tiles = {}

        def issue_kxm(k, h):
            kt = kxm_pool.tile([P, M_CORE // 2], mm_dt, tag="kxm",
                               name=f"kxm_{k}_{h}", bufs=2 * KT)
            eng = nc.scalar if h == 0 else nc.gpsimd
            eng.dma_start(out=kt[:, :],
                          in_=kxm[k * P:(k + 1) * P,
                                  h * (M_CORE // 2):(h + 1) * (M_CORE // 2)])
            kxm_tiles[(k, h)] = kt

        def lhsT(k, m):
            h, off = divmod(m, MT // 2)
            return kxm_tiles[(k, h)][:, off * P:(off + 1) * P]

        sizes0 = [2, 2, 2, 2, 4, 4, 4, 4, 4, 4]
        rhs0, k0 = [], 0
        issue_kxm(0, 0)
        issue_kxm(1, 0)
        for c, sz in enumerate(sizes0):
            rhs0 += issue_chunk(0, c, k0, sz)
            k0 += sz
            for k in range(min(k0 + 2, KT)):
                if (k, 0) not in kxm_tiles:
                    issue_kxm(k, 0)
            for k in range(min(k0 - 8, KT)):
                if (k, 1) not in kxm_tiles:
                    issue_kxm(k, 1)
        for k in range(KT):
            if (k, 0) not in kxm_tiles:
                issue_kxm(k, 0)
        for k in range(KT):
            if (k, 1) not in kxm_tiles:
                issue_kxm(k, 1)

        next_rhs = rhs0
        for nb in range(NB):
            ncols = slice(nb * NW, (nb + 1) * NW)
            rhs_k = next_rhs
            psums = [psum_pool.tile([P, NW], f32, tag="ps", name=f"ps_{nb}_{i}")
                     for i in range(MT)]
            groups = [range(MT // 2), range(MT // 2, MT)] if nb == 0 \
                else [[m] for m in range(MT)]
            for gi, ms in enumerate(groups):
                for k in range(KT):
                    for m in ms:
                        nc.tensor.matmul(
                            psums[m][:, :],
                            lhsT=lhsT(k, m),
                            rhs=rhs_k[k],
                            start=(k == 0), stop=(k == KT - 1))
                if gi == 0 and nb + 1 < NB:
                    next_rhs = issue_chunks(nb + 1, [4] * 8)
                for m in ms:
                    ot = out_pool.tile([P, NW], f32, tag="ot", name=f"ot_{nb}_{m}")
                    nc.vector.tensor_copy(out=ot[:, :], in_=psums[m][:, :])
                    nc.gpsimd.dma_start(
                        out=out[m * P:(m + 1) * P, ncols], in_=ot[:, :])


def _build():
    """Build + compile the 8-core SPMD Bass program once per process."""
    if "nc" in _cache:
        return _cache["nc"]

    import concourse.bacc as bacc
    import concourse.tile as tile
    import concourse.mybir as mybir
    from concourse.kernels.tile_matmul import matmul_tile_kernel

    nc = bacc.Bacc("TRN2", target_bir_lowering=False, debug=False,
                   enable_asserts=bool(os.environ.get("BK_ASSERTS")),
                   num_devices=NCORES)

    def _warmup(tc):
        # Optional HAM-warming matmuls on memset tiles. Off by default
        # for the fp8 path: its first phase is DMA-supply-bound anyway,
        # so the cold (1.2GHz) matmuls usefully slow consumption while
        # kxn streams in; a serialized warmup just delays them.
        n_warm = int(os.environ.get("BK_WARM", "0"))
        if not n_warm:
            return
        from contextlib import ExitStack
        with ExitStack() as ctx:
            wp = ctx.enter_context(tc.tile_pool(name="warm", bufs=1))
            wpp = ctx.enter_context(
                tc.tile_pool(name="warmp", bufs=1, space="PSUM"))
            wdt = mybir.dt.bfloat16
            a = wp.tile([128, 128], wdt)
            b = wp.tile([128, 512], wdt)
            nc.any.memset(a[:, :], 0.0)
            nc.any.memset(b[:, :], 0.0)
            ps = wpp.tile([128, 512], mybir.dt.float32)
            for _ in range(n_warm):
                nc.tensor.matmul(ps[:, :], lhsT=a[:, :], rhs=b[:, :],
                                 start=True, stop=True)

    if IMPL == "fp8":
        f8 = mybir.dt.float8e4
        kxm = nc.dram_tensor("kxm", [KP, 2, 128, 2, M_CORE // 2], f8,
                             kind="ExternalInput").ap()
        kxn = nc.dram_tensor("kxn", [4, SP, 128, 2, D_OUT // 4], f8,
                             kind="ExternalInput").ap()
        kxn00a = nc.dram_tensor("kxn00a", [128, 2, 512], f8,
                                kind="ExternalInput").ap()
        kxn00b = nc.dram_tensor("kxn00b", [128, 2, 512], f8,
                                kind="ExternalInput").ap()
        kxm00 = nc.dram_tensor("kxm00", [128, 2, 256], f8,
                               kind="ExternalInput").ap()
        out = nc.dram_tensor("out", [M_CORE, D_OUT], mybir.dt.float32,
                             kind="ExternalOutput").ap()
        with tile.TileContext(nc) as tc:
            _warmup(tc)
            _fp8_body(nc, tc, kxm, kxn, kxn00a, kxn00b, kxm00, out,
                      mybir)
    else:
        mm_dt = {"f32r": mybir.dt.float32r,
                 "bf16": mybir.dt.bfloat16}[DTYPE]
        kxm = nc.dram_tensor("kxm", [D_IN, M_CORE], mm_dt,
                             kind="ExternalInput").ap()
        kxn = nc.dram_tensor("kxn", [D_IN, D_OUT], mm_dt,
                             kind="ExternalInput").ap()
        out = nc.dram_tensor("out", [M_CORE, D_OUT], mybir.dt.float32,
                             kind="ExternalOutput").ap()
        if IMPL == "custom":
            with tile.TileContext(nc) as tc:
                _warmup(tc)
                _custom_body(nc, tc, kxm, kxn, out, mm_dt, mybir)
        else:
            kw = {}
            if os.environ.get("BK_MAX_K_TILE"):
                kw["MAX_K_TILE_SIZE"] = int(os.environ["BK_MAX_K_TILE"])
            with tile.TileContext(nc) as tc:
                _warmup(tc)
                matmul_tile_kernel(tc, kxm, kxn, out, **kw)
    nc.compile()
    _cache["nc"] = nc
    return nc


def _prep_inputs(x, weight):
    import ml_dtypes
    x2d = np.asarray(x, dtype=np.float32).reshape(M_TOTAL, D_IN)
    if IMPL == "fp8":
        f8 = ml_dtypes.float8_e4m3
        xq = x2d.astype(f8)
        rq = (x2d[:, :CORR] - xq[:, :CORR].astype(np.float32)).astype(f8)
        swt = np.sign(weight, dtype=np.float32).T.astype(f8)
        # kxn [4, SP, 128, 2, D_OUT//4]: (q, sp, p, i, n) =
        #   swt[sp*256 + i*128 + p, q*1024 + n]
        kxn = np.ascontiguousarray(
            swt.reshape(SP, 2, 128, 4, D_OUT // 4).transpose(3, 0, 2, 1, 4))
        in_maps = []
        for c in range(NCORES):
            rows = slice(c * M_CORE, (c + 1) * M_CORE)
            kxm2d = np.concatenate(
                [np.ascontiguousarray(xq[rows].T),
                 np.ascontiguousarray(rq[rows].T)], axis=0)
            # kxm [KP, 2, 128, 2, M_CORE//2]: (k, mh, p, i, mm) =
            #   kxm2d[k*256 + i*128 + p, mh*512 + mm]
            kxm = np.ascontiguousarray(
                kxm2d.reshape(KP, 2, 128, 2, M_CORE // 2)
                .transpose(0, 3, 2, 1, 4))
            in_maps.append({
                "kxm": kxm, "kxn": kxn,
                "kxn00a": np.ascontiguousarray(kxn[0, 0][:, :, :512]),
                "kxn00b": np.ascontiguousarray(kxn[0, 0][:, :, 512:]),
                "kxm00": np.ascontiguousarray(kxm[0, 0][:, :, :256]),
            })
        return in_maps
    if DTYPE == "bf16":
        np_dt = ml_dtypes.bfloat16
    else:
        np_dt = np.float32
    kxn = np.ascontiguousarray(
        np.sign(weight, dtype=np.float32).T.astype(np_dt))
    in_maps = []
    for c in range(NCORES):
        kxm = np.ascontiguousarray(
            x2d[c * M_CORE:(c + 1) * M_CORE].T.astype(np_dt))
        in_maps.append({"kxm": kxm, "kxn": kxn})
    return in_maps


def _run(x, weight, bias, trace=False):
    from concourse.bass_utils import run_bass_kernel_spmd

    nc = _build()
    in_maps = _prep_inputs(x, weight)
    res = run_bass_kernel_spmd(nc, in_maps, core_ids=list(range(NCORES)),
                               trace=trace)
    out = np.concatenate([res.results[c]["out"] for c in range(NCORES)],
                         axis=0)
    bias = np.asarray(bias, dtype=np.float32)
    if np.any(bias):
        out += bias
    return out.reshape(B, S, D_OUT), res


def kernel(x, weight, bias):
    out, _ = _run(x, weight, bias, trace=False)
    return out


# revision 24
# speedup vs baseline: 1.0139x; 1.0139x over previous
"""BitNet-style row-parallel linear on 8 TRN2 NeuronCores.

Reference computes: out[b,s,o] = sum_d x[b,s,d] * sign(w[o,d]) + bias[o]
  x: [4, 2048, 4096] f32, w: [4096, 4096] f32, bias: [4096] f32.

Strategy: data-parallel over the 8192 (b*s) rows — each of the 8 cores
computes a 1024-row slice of the output against the full binarized
weight. No collective needed; shards concatenate to the full output.

Default IMPL "fp8": the GEMM runs in fp8 e4m3 with DoubleRow perf mode
(2 fp8 MACs per PE cell per cycle; each N=512 matmul covers K=256 in
the same ~216ns a bf16 matmul spends on K=128 — LDWEIGHTS hides fully
under the moving stream). Weights are exactly +-1 in e4m3 so all
quantization error comes from x. Pure-e4m3 error (2.12e-2 rel) just
misses the 2e-2 gate, so the K dimension is extended with residual
correction columns: for the first CORR=768 input features,
fp8(x - fp8(x)) is appended as extra K rows whose +-1 weight rows
duplicate the originals, so the SBUF weight tiles are simply reused —
no extra weight HBM traffic. K' = 4864, measured rel err 1.884e-2
(deterministic: fixed seed, fp32 PSUM accumulation, bit-identical
across runs; numpy emulation matches HW to 0.02%).

TensorE consumes both operands K-major with the DoubleRow pair layout
[128 partitions, 2, free]: partition p, pair i holds logical k row
tile*256 + i*128 + p. The host pre-tiles both operands into exactly
this layout so every DMA is a flat contiguous per-partition copy.

Schedule (the PE stream measures gap-free at the 216ns/MM instruction
floor; total ~281us vs 464us for the bf16 baseline):
 - kxn (16MB) becomes fully SBUF-resident as 64 quarter-tiles;
   kxm streams in m-halves on separate DMA rings, ordered exactly as
   consumed (issue order != consumption order cost 11us of stalls).
 - Phase 1 (while kxn streams): nb {0,1} for m-quads, k outer in an
   order that interleaves residual k-tiles (whose kxn tiles arrived
   first) — paces fresh-byte demand under the ~345GB/s DMA supply.
 - Phase 2: per m-tile, per PSUM bank (nb 2..7), dense k loop;
   evictions (vector/scalar alternating) and output DMAs (3 rings)
   pipeline behind the next bank's matmuls.
No warmup: phase 1's first ~8 cold (1.2GHz) matmuls usefully slow
consumption while the HAM clock gate opens and kxn streams in.
"""

import numpy as np

B, S, D_IN, D_OUT = 4, 2048, 4096, 4096
NCORES = 8
M_TOTAL = B * S
M_CORE = M_TOTAL // NCORES

import os

_cache = {}

# fp8 config: pair-tiles cover 256 logical k each.
CORR = int(os.environ.get("BK_CORR", "768"))   # residual-corrected columns
KP = (D_IN + CORR) // 256                       # kxm pair-tiles
SP = D_IN // 256                                # kxn base pair-tiles (16)
assert (D_IN + CORR) % 256 == 0 and CORR // 256 <= SP

DTYPE = os.environ.get("BK_DTYPE", "bf16")      # for legacy bf16 impls
IMPL = os.environ.get("BK_IMPL", "fp8")


def _fp8_body(nc, tc, kxm, kxn, out, mybir):
    P = 128
    MT = M_CORE // P        # 8 m tiles
    NW = 512
    NB = D_OUT // NW        # 8 n blocks
    RES = CORR // 256       # residual pair-tiles (reuse kxn tiles 0..RES-1)
    f32 = mybir.dt.float32
    f8 = mybir.dt.float8e4
    DR = mybir.MatmulPerfMode.DoubleRow

    from contextlib import ExitStack
    with ExitStack() as ctx:
        kxn_pool = ctx.enter_context(tc.tile_pool(name="kxn", bufs=4 * SP))
        kxm_pool = ctx.enter_context(tc.tile_pool(name="kxm", bufs=2 * KP))
        psum_pool = ctx.enter_context(
            tc.tile_pool(name="psum", bufs=8, space="PSUM"))
        out_pool = ctx.enter_context(tc.tile_pool(name="outp", bufs=8))

        # kxn: 4 n-quarters x 16 base pair-tiles, all SBUF-resident
        # (2KB/partition each). Residual k rows reuse tiles 0..RES-1.
        # Issue order = first-needed order: quarter 0 first, so the
        # warm-up phase only waits on 256KB-granularity arrivals.
        kxn_tiles = {}
        for q in range(4):
            for sp in range(SP):
                t = kxn_pool.tile([P, 2, D_OUT // 4], f8, tag="kxn",
                                  name=f"kxn_{q}_{sp}", bufs=4 * SP)
                nc.sync.dma_start(out=t, in_=kxn[q, sp])
                kxn_tiles[(q, sp)] = t
        # Warm-up k order: residual tiles (which reuse the
        # earliest-arriving kxn tiles) interleaved among the first base
        # tiles to pace consumption of fresh DMA bytes.
        korder = []
        for k in range(KP):
            korder.append(k)
            if k < RES:
                korder.append(SP + k)
        korder += [k for k in range(RES, SP) if k not in korder]
        korder = korder[:KP]
        assert sorted(korder) == list(range(KP))

        # kxm: m-halves so the warm-up phase only pulls the rows it
        # uses. mh0 is issued in korder (the order P1a consumes it);
        # mh1 rides the otherwise-idle gpsimd ring so it doesn't
        # compete with mh0+kxn during the supply-bound warm-up.
        kxm_tiles = {}
        for mh, ks, eng in ((0, korder, nc.scalar), (1, range(KP), nc.gpsimd)):
            for k in ks:
                t = kxm_pool.tile([P, 2, M_CORE // 2], f8, tag="kxm",
                                  name=f"kxm_{k}_{mh}", bufs=2 * KP)
                eng.dma_start(out=t, in_=kxm[k, mh])
                kxm_tiles[(k, mh)] = t

        def lhsT(k, m):
            mh, off = divmod(m, MT // 2)
            return kxm_tiles[(k, mh)][:, :, off * P:(off + 1) * P]

        def rhs(sp, nb):
            q, j = divmod(nb, 2)
            return kxn_tiles[(q, sp)][:, :, j * NW:(j + 1) * NW]

        def evict(psum_t, m, nb, j):
            ot = out_pool.tile([P, NW], f32, tag="ot",
                               name=f"ot_{m}_{nb}", bufs=8)
            if j % 2 == 0:
                nc.vector.tensor_copy(out=ot[:, :], in_=psum_t[:, :])
            else:
                nc.scalar.copy(out=ot[:, :], in_=psum_t[:, :])
            deng = (nc.gpsimd, nc.sync, nc.scalar)[j % 3]
            deng.dma_start(
                out=out[m * P:(m + 1) * P, nb * NW:(nb + 1) * NW],
                in_=ot[:, :])

        # Phase 1: nb {0,1} for all m, in two m-quads. 2x weight reuse,
        # paced so kxn/kxm DMA keeps up; P1b re-uses kxn while the rest
        # of kxn streams in behind it.
        for mq in range(2):
            ms = range(mq * 4, mq * 4 + 4)
            psums = {(m, nb): psum_pool.tile(
                [P, NW], f32, tag="ps", name=f"ps1_{m}_{nb}")
                for m in ms for nb in range(2)}
            for ki, k in enumerate(korder if mq == 0 else range(KP)):
                sp = k if k < SP else k - SP
                for m in ms:
                    lt = lhsT(k, m)
                    for nb in range(2):
                        nc.tensor.matmul(
                            psums[(m, nb)][:, :], lhsT=lt, rhs=rhs(sp, nb),
                            start=(ki == 0), stop=(ki == KP - 1),
                            perf_mode=DR)
            for j, (m, nb) in enumerate(psums):
                evict(psums[(m, nb)], m, nb, j)

        # Phase 2: per m-tile, nb {2..7}, one bank at a time with its
        # own k-loop (LDWEIGHTS hides under every 512-wide matmul, so
        # reuse order is free): each bank's eviction+store pipelines
        # behind the next bank's matmuls, leaving only a ~2us tail.
        for m in range(MT):
            psums = [psum_pool.tile([P, NW], f32, tag="ps",
                                    name=f"ps2_{m}_{j}")
                     for j in range(NB - 2)]
            for j in range(NB - 2):
                for k in range(KP):
                    sp = k if k < SP else k - SP
                    nc.tensor.matmul(
                        psums[j][:, :], lhsT=lhsT(k, m), rhs=rhs(sp, j + 2),
                        start=(k == 0), stop=(k == KP - 1),
                        perf_mode=DR)
                evict(psums[j], m, j + 2, j)


def _custom_body(nc, tc, kxm, kxn, out, mm_dt, mybir):
    """Legacy bf16 path: x^T SBUF-resident; sign(w)^T streams through."""
    P = 128
    KT = D_IN // P          # 32 k tiles
    MT = M_CORE // P        # 8 m tiles
    NW = 512
    NB = D_OUT // NW        # 8 n blocks
    f32 = mybir.dt.float32

    from contextlib import ExitStack
    with ExitStack() as ctx:
        kxm_pool = ctx.enter_context(tc.tile_pool(name="kxm", bufs=1))
        kxn_pool = ctx.enter_context(tc.tile_pool(name="kxn", bufs=9))
        psum_pool = ctx.enter_context(
            tc.tile_pool(name="psum", bufs=8, space="PSUM"))
        out_pool = ctx.enter_context(tc.tile_pool(name="outp", bufs=8))

        def issue_chunk(nb, c, k0, sz):
            t = kxn_pool.tile([P, sz, NW], mm_dt, tag="kxn",
                              name=f"kxn_{nb}_{c}", bufs=24)
            src = kxn[k0 * P:(k0 + sz) * P, nb * NW:(nb + 1) * NW]
            nc.sync.dma_start(
                out=t, in_=src.rearrange("(ko ki) n -> ki ko n", ki=P))
            return [t[:, i, :] for i in range(sz)]

        def issue_chunks(nb, sizes):
            rhs, k0 = [], 0
            for c, sz in enumerate(sizes):
                rhs += issue_chunk(nb, c, k0, sz)
                k0 += sz
            return rhs

        kxm_tiles = {}

        def issue_kxm(k, h):
            kt = kxm_pool.tile([P, M_CORE // 2], mm_dt, tag="kxm",
                               name=f"kxm_{k}_{h}", bufs=2 * KT)
            eng = nc.scalar if h == 0 else nc.gpsimd
            eng.dma_start(out=kt[:, :],
                          in_=kxm[k * P:(k + 1) * P,
                                  h * (M_CORE // 2):(h + 1) * (M_CORE // 2)])
            kxm_tiles[(k, h)] = kt

        def lhsT(k, m):
            h, off = divmod(m, MT // 2)
            return kxm_tiles[(k, h)][:, off * P:(off + 1) * P]

        sizes0 = [2, 2, 2, 2, 4, 4, 4, 4, 4, 4]
        rhs0, k0 = [], 0
        issue_kxm(0, 0)
        issue_kxm(1, 0)
        for c, sz in enumerate(sizes0):
            rhs0 += issue_chunk(0, c, k0, sz)
            k0 += sz
            for k in range(min(k0 + 2, KT)):
                if (k, 0) not in kxm_tiles:
                    issue_kxm(k, 0)
            for k in range(min(k0 - 8, KT)):
                if (k, 1) not in kxm_tiles:
                    issue_kxm(k, 1)
        for k in range(KT):
            if (k, 0) not in kxm_tiles:
                issue_kxm(k, 0)
        for k in range(KT):
            if (k, 1) not in kxm_tiles:
                issue_kxm(k, 1)

        next_rhs = rhs0
        for nb in range(NB):
            ncols = slice(nb * NW, (nb + 1) * NW)
            rhs_k = next_rhs
            psums = [psum_pool.tile([P, NW], f32, tag="ps", name=f"ps_{nb}_{i}")
                     for i in range(MT)]
            groups = [range(MT // 2), range(MT // 2, MT)] if nb == 0 \
                else [[m] for m in range(MT)]
            for gi, ms in enumerate(groups):
                for k in range(KT):
                    for m in ms:
                        nc.tensor.matmul(
                            psums[m][:, :],
                            lhsT=lhsT(k, m),
                            rhs=rhs_k[k],
                            start=(k == 0), stop=(k == KT - 1))
                if gi == 0 and nb + 1 < NB:
                    next_rhs = issue_chunks(nb + 1, [4] * 8)
                for m in ms:
                    ot = out_pool.tile([P, NW], f32, tag="ot", name=f"ot_{nb}_{m}")
                    nc.vector.tensor_copy(out=ot[:, :], in_=psums[m][:, :])
                    nc.gpsimd.dma_start(
                        out=out[m * P:(m + 1) * P, ncols], in_=ot[:, :])


def _build():
    """Build + compile the 8-core SPMD Bass program once per process."""
    if "nc" in _cache:
        return _cache["nc"]

    import concourse.bacc as bacc
    import concourse.tile as tile
    import concourse.mybir as mybir
    from concourse.kernels.tile_matmul import matmul_tile_kernel

    nc = bacc.Bacc("TRN2", target_bir_lowering=False, debug=False,
                   enable_asserts=bool(os.environ.get("BK_ASSERTS")),
                   num_devices=NCORES)

    def _warmup(tc):
        # Optional HAM-warming matmuls on memset tiles. Off by default
        # for the fp8 path: its first phase is DMA-supply-bound anyway,
        # so the cold (1.2GHz) matmuls usefully slow consumption while
        # kxn streams in; a serialized warmup just delays them.
        n_warm = int(os.environ.get("BK_WARM", "0"))
        if not n_warm:
            return
        from contextlib import ExitStack
        with ExitStack() as ctx:
            wp = ctx.enter_context(tc.tile_pool(name="warm", bufs=1))
            wpp = ctx.enter_context(
                tc.tile_pool(name="warmp", bufs=1, space="PSUM"))
            wdt = mybir.dt.bfloat16
            a = wp.tile([128, 128], wdt)
            b = wp.tile([128, 512], wdt)
            nc.any.memset(a[:, :], 0.0)
            nc.any.memset(b[:, :], 0.0)
            ps = wpp.tile([128, 512], mybir.dt.float32)
            for _ in range(n_warm):
                nc.tensor.matmul(ps[:, :], lhsT=a[:, :], rhs=b[:, :],
                                 start=True, stop=True)

    if IMPL == "fp8":
        f8 = mybir.dt.float8e4
        kxm = nc.dram_tensor("kxm", [KP, 2, 128, 2, M_CORE // 2], f8,
                             kind="ExternalInput").ap()
        kxn = nc.dram_tensor("kxn", [4, SP, 128, 2, D_OUT // 4], f8,
                             kind="ExternalInput").ap()
        out = nc.dram_tensor("out", [M_CORE, D_OUT], mybir.dt.float32,
                             kind="ExternalOutput").ap()
        with tile.TileContext(nc) as tc:
            _warmup(tc)
            _fp8_body(nc, tc, kxm, kxn, out, mybir)
    else:
        mm_dt = {"f32r": mybir.dt.float32r,
                 "bf16": mybir.dt.bfloat16}[DTYPE]
        kxm = nc.dram_tensor("kxm", [D_IN, M_CORE], mm_dt,
                             kind="ExternalInput").ap()
        kxn = nc.dram_tensor("kxn", [D_IN, D_OUT], mm_dt,
                             kind="ExternalInput").ap()
        out = nc.dram_tensor("out", [M_CORE, D_OUT], mybir.dt.float32,
                             kind="ExternalOutput").ap()
        if IMPL == "custom":
            with tile.TileContext(nc) as tc:
                _warmup(tc)
                _custom_body(nc, tc, kxm, kxn, out, mm_dt, mybir)
        else:
            kw = {}
            if os.environ.get("BK_MAX_K_TILE"):
                kw["MAX_K_TILE_SIZE"] = int(os.environ["BK_MAX_K_TILE"])
            with tile.TileContext(nc) as tc:
                _warmup(tc)
                matmul_tile_kernel(tc, kxm, kxn, out, **kw)
    nc.compile()
    _cache["nc"] = nc
    return nc


def _prep_inputs(x, weight):
    import ml_dtypes
    x2d = np.asarray(x, dtype=np.float32).reshape(M_TOTAL, D_IN)
    if IMPL == "fp8":
        f8 = ml_dtypes.float8_e4m3
        xq = x2d.astype(f8)
        rq = (x2d[:, :CORR] - xq[:, :CORR].astype(np.float32)).astype(f8)
        swt = np.sign(weight, dtype=np.float32).T.astype(f8)
        # kxn [4, SP, 128, 2, D_OUT//4]: (q, sp, p, i, n) =
        #   swt[sp*256 + i*128 + p, q*1024 + n]
        kxn = np.ascontiguousarray(
            swt.reshape(SP, 2, 128, 4, D_OUT // 4).transpose(3, 0, 2, 1, 4))
        in_maps = []
        for c in range(NCORES):
            rows = slice(c * M_CORE, (c + 1) * M_CORE)
            kxm2d = np.concatenate(
                [np.ascontiguousarray(xq[rows].T),
                 np.ascontiguousarray(rq[rows].T)], axis=0)
            # kxm [KP, 2, 128, 2, M_CORE//2]: (k, mh, p, i, mm) =
            #   kxm2d[k*256 + i*128 + p, mh*512 + mm]
            kxm = np.ascontiguousarray(
                kxm2d.reshape(KP, 2, 128, 2, M_CORE // 2)
                .transpose(0, 3, 2, 1, 4))
            in_maps.append({"kxm": kxm, "kxn": kxn})
        return in_maps
    if DTYPE == "bf16":
        np_dt = ml_dtypes.bfloat16
    else:
        np_dt = np.float32
    kxn = np.ascontiguousarray(
        np.sign(weight, dtype=np.float32).T.astype(np_dt))
    in_maps = []
    for c in range(NCORES):
        kxm = np.ascontiguousarray(
            x2d[c * M_CORE:(c + 1) * M_CORE].T.astype(np_dt))
        in_maps.append({"kxm": kxm, "kxn": kxn})
    return in_maps


def _run(x, weight, bias, trace=False):
    from concourse.bass_utils import run_bass_kernel_spmd

    nc = _build()
    in_maps = _prep_inputs(x, weight)
    res = run_bass_kernel_spmd(nc, in_maps, core_ids=list(range(NCORES)),
                               trace=trace)
    out = np.concatenate([res.results[c]["out"] for c in range(NCORES)],
                         axis=0)
    bias = np.asarray(bias, dtype=np.float32)
    if np.any(bias):
        out += bias
    return out.reshape(B, S, D_OUT), res


def kernel(x, weight, bias):
    out, _ = _run(x, weight, bias, trace=False)
    return out
